# revision 32
# baseline (speedup 1.0000x reference)
"""Trainium2 Bass kernel for nn_Diversity6 (pairwise-correlation diversity loss).

Math (per sample row b, per model m):
    e_m = exp(x_m / T)                      (softmax numerator; inputs are small,
                                             no max-subtraction needed)
    p_m = e_m / sum(e_m)
    u_m = (p_m - mean(p_m)) / ||p_m - mean(p_m)||
        = (e_m - mean(e_m)) / sqrt(C * var(e_m))      (sum(e) cancels!)
        = alpha_m * e_m + b_m
    with alpha_m = 1/sqrt(C*var(e_m)), b_m = -mean(e_m)*alpha_m.

    sum over all ordered pairs of corr(u_m, u_n) = ||sum_m u_m||^2, so
    d_b = (||s_b||^2 - M)/2 with s = sum_m u_m, and
    loss = SCALE * mean_b d_b.

Sharding: data-parallel over the batch dim, 512 rows per core on 8 cores.
Each core returns per-(partition, row-tile) values of ||s||^2; the host sums
them and applies the affine to produce the scalar loss.

Numerics: shifted moments (w = e - 1) avoid the catastrophic cancellation of
Q - S^2/C at |S|~1000 in fp32, and one Newton step repairs the ~2e-4 relative
error of the Ln/Exp LUT path for alpha = rsqrt(tvar).
"""

import math
from contextlib import ExitStack

import numpy as np

import concourse.bass as bass
import concourse.mybir as mybir
import concourse.tile as tile
from concourse import bacc
from concourse.bass_utils import run_bass_kernel_spmd

N_CORES = 8
B_TOTAL = 4096
C = 1000
M = 6
P = 128
RPC = B_TOTAL // N_CORES  # rows per core = 512
NT = RPC // P             # row-tiles per core = 4
T_INV = 1.0 / 20.0
SCALE = 0.3
NEG_HALF_LN_C = -0.5 * math.log(float(C))

F32 = mybir.dt.float32
AF = mybir.ActivationFunctionType
OP = mybir.AluOpType

TRACE = False
DEBUG = False
LAST_RESULT = None
N_QACT = 3  # how many models' qw runs on ACT (Square) vs DVE (stt)


def _body(ctx, tc, nc, xs, out, dbg=None):
    xv = [x.rearrange("(t p) c -> p t c", p=P) for x in xs]

    xpool = ctx.enter_context(tc.tile_pool(name="x", bufs=1))
    bnpool = ctx.enter_context(tc.tile_pool(name="bn", bufs=3))
    mvpool = ctx.enter_context(tc.tile_pool(name="mv", bufs=2))
    stpool = ctx.enter_context(tc.tile_pool(name="st", bufs=2))
    accpool = ctx.enter_context(tc.tile_pool(name="acc", bufs=3))
    sqpool = ctx.enter_context(tc.tile_pool(name="sq", bufs=2, space="PSUM"))
    opool = ctx.enter_context(tc.tile_pool(name="o", bufs=1))

    # Resident model tiles; 6 x 16KB/partition = 96KB/partition of SBUF.
    xt = [xpool.tile([P, NT, C], F32, tag=f"x{m}", name=f"x{m}sb") for m in range(M)]
    # Two 1MB DMAs per model, issued in the order compute consumes them.
    for h in range(2):
        for m in range(M):
            nc.sync.dma_start(
                xt[m][:, 2 * h : 2 * h + 2, :], xv[m][:, 2 * h : 2 * h + 2, :]
            )

    # Constant -1.0 bias column for the ACT Square(e-1) passes.
    negone = stpool.tile([P, 1], F32, tag="negone")
    nc.vector.memset(negone[:, :], -1.0)

    ssq = opool.tile([P, NT], F32)
    for t in range(NT):
        # Shifted moments, w = e - 1 (small!), to avoid the catastrophic
        # cancellation of Q - S^2/C at |S|~1000 in fp32:
        #   sw = sum(w);  qw = sum(w^2);  tvar = C*var(e) = qw - sw^2/C.
        sw = mvpool.tile([P, M], F32, tag="sw")
        qw = mvpool.tile([P, M], F32, tag="qw")
        for m in range(M):
            e = xt[m][:, t, :]  # [P, C]
            nc.scalar.activation(e, e, AF.Exp, scale=T_INV)
            wdump = bnpool.tile([P, 1], F32, tag="wdump")
            nc.vector.tensor_scalar(
                wdump.broadcast_to((P, C)), e, -1.0, 0.0, OP.add, OP.add,
                accum_out=sw[:, m : m + 1],
            )
            if m < M - N_QACT:
                # qw holds sum((e-1)*e) = qw + sw for now; fixed below.
                esq = bnpool.tile([P, 1], F32, tag="esq")
                nc.vector.scalar_tensor_tensor(
                    esq.broadcast_to((P, C)), e, -1.0, e, OP.add, OP.mult,
                    accum_out=qw[:, m : m + 1],
                )
            else:
                # ACT path: Square(e - 1) accumulated = sum(w^2) directly.
                sqs = sqpool.tile([P, C], F32, tag="sqs")
                nc.scalar.activation(
                    sqs[:, :], e, AF.Square, bias=negone[:, :],
                    accum_out=qw[:, m : m + 1],
                )
        nq = M - N_QACT
        if nq > 0:
            # DVE-path columns hold sum(w*e) = qw + sw; subtract sw.
            nc.vector.tensor_sub(qw[:, 0:nq], qw[:, 0:nq], sw[:, 0:nq])

        # tvar = qw - sw^2/C
        tvar = stpool.tile([P, M], F32, tag="tvar")
        nc.vector.scalar_tensor_tensor(
            tvar[:, :], sw[:, :], -1.0 / C, sw[:, :], OP.mult, OP.mult
        )
        nc.vector.tensor_add(tvar[:, :], tvar[:, :], qw[:, :])
        lnv = stpool.tile([P, M], F32, tag="lnv")
        nc.scalar.activation(lnv[:, :], tvar[:, :], AF.Ln)
        alpha0 = stpool.tile([P, M], F32, tag="alpha0")
        nc.scalar.activation(alpha0[:, :], lnv[:, :], AF.Exp, scale=-0.5)
        # One Newton step for rsqrt: alpha = alpha0*(1.5 - 0.5*tvar*alpha0^2).
        # The Ln/Exp LUT path is only ~2e-4 accurate; this makes it fp32-exact.
        nwt = stpool.tile([P, M], F32, tag="nwt")
        nc.vector.tensor_mul(nwt[:, :], alpha0[:, :], alpha0[:, :])
        nc.vector.tensor_mul(nwt[:, :], nwt[:, :], tvar[:, :])
        nc.vector.tensor_scalar(nwt[:, :], nwt[:, :], -0.5, 1.5, OP.mult, OP.add)
        alpha = stpool.tile([P, M], F32, tag="alpha")
        nc.vector.tensor_mul(alpha[:, :], alpha0[:, :], nwt[:, :])
        # b_m = -(S_m/C)*alpha_m = -(1 + sw_m/C)*alpha_m
        zz = stpool.tile([P, M], F32, tag="zz")
        nc.vector.tensor_scalar(zz[:, :], sw[:, :], 1.0 / C, 1.0, OP.mult, OP.add)
        bvals = stpool.tile([P, M], F32, tag="b")
        nc.vector.scalar_tensor_tensor(
            bvals[:, :], zz[:, :], -1.0, alpha[:, :], OP.mult, OP.mult
        )
        # Bias for the final Square: sum of b_m for m>=1 (b_0 folded into acc).
        bsum = stpool.tile([P, 1], F32, tag="bsum")
        nc.vector.reduce_sum(bsum[:, :], bvals[:, 1:M], axis=mybir.AxisListType.X)

        # s-hat accumulation chain
        acc = accpool.tile([P, C], F32, tag="acc")
        nc.vector.tensor_scalar(
            acc[:, :], xt[0][:, t, :], alpha[:, 0:1], bvals[:, 0:1], OP.mult, OP.add
        )
        for m in range(1, M):
            nacc = accpool.tile([P, C], F32, tag="acc")
            nc.vector.scalar_tensor_tensor(
                nacc[:, :], xt[m][:, t, :], alpha[:, m : m + 1], acc[:, :],
                OP.mult, OP.add,
            )
            acc = nacc

        sq = sqpool.tile([P, C], F32)
        nc.scalar.activation(
            sq[:, :], acc[:, :], AF.Square, bias=bsum[:, :],
            accum_out=ssq[:, t : t + 1],
        )
        if dbg is not None and t == 0:
            d_sv, d_qv, d_al, d_acc = dbg
            nc.sync.dma_start(d_sv[:, :], sw[:, :])
            nc.sync.dma_start(d_qv[:, :], tvar[:, :])
            nc.sync.dma_start(d_al[:, :], alpha[:, :])
            nc.sync.dma_start(d_acc[:, :], acc[:, :])

    nc.sync.dma_start(out[:, :], ssq[:, :])


def build_program(debug=False):
    nc = bacc.Bacc()
    xs = [
        nc.declare_dram_parameter(f"x{m}", [RPC, C], F32, isOutput=False)
        for m in range(M)
    ]
    out = nc.declare_dram_parameter("out", [P, NT], F32, isOutput=True)
    dbg = None
    if debug:
        dbg = (
            nc.declare_dram_parameter("d_sv", [P, M], F32, isOutput=True),
            nc.declare_dram_parameter("d_qv", [P, M], F32, isOutput=True),
            nc.declare_dram_parameter("d_al", [P, M], F32, isOutput=True),
            nc.declare_dram_parameter("d_acc", [P, C], F32, isOutput=True),
        )
    with tile.TileContext(nc) as tc:
        with ExitStack() as ctx:
            _body(ctx, tc, nc, xs, out, dbg)
    nc.compile()
    return nc


_prog = None


def kernel(**inputs):
    global _prog, LAST_RESULT
    xs_full = [
        np.ascontiguousarray(np.asarray(inputs[f"outputs{m + 1}"], dtype=np.float32))
        for m in range(M)
    ]
    if _prog is None:
        _prog = build_program(debug=DEBUG)
    core_ids = list(range(N_CORES))
    in_maps = [
        {f"x{m}": xs_full[m][k * RPC : (k + 1) * RPC] for m in range(M)}
        for k in core_ids
    ]
    res = run_bass_kernel_spmd(_prog, in_maps, core_ids, trace=TRACE)
    LAST_RESULT = res
    total = 0.0
    for r in res.results:
        total += np.asarray(r["out"], dtype=np.float64).sum()
    loss = SCALE * 0.5 * (total / B_TOTAL - M)
    return np.asarray(loss, dtype=np.float32)


# revision 34
# speedup vs baseline: 1.0421x; 1.0421x over previous
"""Trainium2 Bass kernel for nn_Diversity6 (pairwise-correlation diversity loss).

Math (per sample row b, per model m):
    e_m = exp(x_m / T)                      (softmax numerator; inputs are small,
                                             no max-subtraction needed)
    p_m = e_m / sum(e_m)
    u_m = (p_m - mean(p_m)) / ||p_m - mean(p_m)||
        = (e_m - mean(e_m)) / sqrt(C * var(e_m))      (sum(e) cancels!)
        = alpha_m * e_m + b_m
    with alpha_m = 1/sqrt(C*var(e_m)), b_m = -mean(e_m)*alpha_m.

    sum over all ordered pairs of corr(u_m, u_n) = ||sum_m u_m||^2, so
    d_b = (||s_b||^2 - M)/2 with s = sum_m u_m, and
    loss = SCALE * mean_b d_b.

Sharding: data-parallel over the batch dim, 512 rows per core on 8 cores.
Each core returns per-(partition, row-tile) values of ||s||^2; the host sums
them and applies the affine to produce the scalar loss.

Numerics: shifted moments (w = e - 1) avoid the catastrophic cancellation of
Q - S^2/C at |S|~1000 in fp32, and one Newton step repairs the ~2e-4 relative
error of the Ln/Exp LUT path for alpha = rsqrt(tvar).
"""

import math
from contextlib import ExitStack

import numpy as np

import concourse.bass as bass
import concourse.mybir as mybir
import concourse.tile as tile
from concourse import bacc
from concourse.bass_utils import run_bass_kernel_spmd

N_CORES = 8
B_TOTAL = 4096
C = 1000
M = 6
P = 128
RPC = B_TOTAL // N_CORES  # rows per core = 512
NT = RPC // P             # row-tiles per core = 4
T_INV = 1.0 / 20.0
SCALE = 0.3
NEG_HALF_LN_C = -0.5 * math.log(float(C))

F32 = mybir.dt.float32
AF = mybir.ActivationFunctionType
OP = mybir.AluOpType

TRACE = False
DEBUG = False
LAST_RESULT = None
N_QACT = 4  # how many models' qw runs on ACT (Square) vs DVE (stt)


def _body(ctx, tc, nc, xs, out, dbg=None):
    xv = [x.rearrange("(t p) c -> p t c", p=P) for x in xs]

    xpool = ctx.enter_context(tc.tile_pool(name="x", bufs=1))
    bnpool = ctx.enter_context(tc.tile_pool(name="bn", bufs=3))
    mvpool = ctx.enter_context(tc.tile_pool(name="mv", bufs=2))
    stpool = ctx.enter_context(tc.tile_pool(name="st", bufs=2))
    accpool = ctx.enter_context(tc.tile_pool(name="acc", bufs=3))
    sqpool = ctx.enter_context(tc.tile_pool(name="sq", bufs=2, space="PSUM"))
    opool = ctx.enter_context(tc.tile_pool(name="o", bufs=1))

    # Resident model tiles; 6 x 16KB/partition = 96KB/partition of SBUF.
    xt = [xpool.tile([P, NT, C], F32, tag=f"x{m}", name=f"x{m}sb") for m in range(M)]
    # Two 1MB DMAs per model, issued in the order compute consumes them.
    for h in range(2):
        for m in range(M):
            nc.sync.dma_start(
                xt[m][:, 2 * h : 2 * h + 2, :], xv[m][:, 2 * h : 2 * h + 2, :]
            )

    # Constant -1.0 bias column for the ACT Square(e-1) passes.
    negone = stpool.tile([P, 1], F32, tag="negone")
    nc.vector.memset(negone[:, :], -1.0)

    ssq = opool.tile([P, NT], F32)
    for t in range(NT):
        # Shifted moments, w = e - 1 (small!), to avoid the catastrophic
        # cancellation of Q - S^2/C at |S|~1000 in fp32:
        #   sw = sum(w);  qw = sum(w^2);  tvar = C*var(e) = qw - sw^2/C.
        sw = mvpool.tile([P, M], F32, tag="sw")
        qw = mvpool.tile([P, M], F32, tag="qw")
        for m in range(M):
            e = xt[m][:, t, :]  # [P, C]
            nc.scalar.activation(e, e, AF.Exp, scale=T_INV)
            # Real (non-broadcast) out so the single-src tensor_scalar can
            # hit the 2x_2P DVE perf mode.
            wdump = bnpool.tile([P, C], F32, tag="wdump")
            nc.vector.tensor_scalar(
                wdump[:, :], e, -1.0, 0.0, OP.add, OP.add,
                accum_out=sw[:, m : m + 1],
            )
            if m < M - N_QACT:
                # qw holds sum((e-1)*e) = qw + sw for now; fixed below.
                esq = bnpool.tile([P, 1], F32, tag="esq")
                nc.vector.scalar_tensor_tensor(
                    esq.broadcast_to((P, C)), e, -1.0, e, OP.add, OP.mult,
                    accum_out=qw[:, m : m + 1],
                )
            else:
                # ACT path: Square(e - 1) accumulated = sum(w^2) directly.
                sqs = sqpool.tile([P, C], F32, tag="sqs")
                nc.scalar.activation(
                    sqs[:, :], e, AF.Square, bias=negone[:, :],
                    accum_out=qw[:, m : m + 1],
                )
        nq = M - N_QACT
        if nq > 0:
            # DVE-path columns hold sum(w*e) = qw + sw; subtract sw.
            nc.vector.tensor_sub(qw[:, 0:nq], qw[:, 0:nq], sw[:, 0:nq])

        # tvar = qw - sw^2/C
        tvar = stpool.tile([P, M], F32, tag="tvar")
        nc.vector.scalar_tensor_tensor(
            tvar[:, :], sw[:, :], -1.0 / C, sw[:, :], OP.mult, OP.mult
        )
        nc.vector.tensor_add(tvar[:, :], tvar[:, :], qw[:, :])
        lnv = stpool.tile([P, M], F32, tag="lnv")
        nc.scalar.activation(lnv[:, :], tvar[:, :], AF.Ln)
        alpha0 = stpool.tile([P, M], F32, tag="alpha0")
        nc.scalar.activation(alpha0[:, :], lnv[:, :], AF.Exp, scale=-0.5)
        # One Newton step for rsqrt: alpha = alpha0*(1.5 - 0.5*tvar*alpha0^2).
        # The Ln/Exp LUT path is only ~2e-4 accurate; this makes it fp32-exact.
        nwt = stpool.tile([P, M], F32, tag="nwt")
        nc.vector.tensor_mul(nwt[:, :], alpha0[:, :], alpha0[:, :])
        nc.vector.tensor_mul(nwt[:, :], nwt[:, :], tvar[:, :])
        nc.vector.tensor_scalar(nwt[:, :], nwt[:, :], -0.5, 1.5, OP.mult, OP.add)
        alpha = stpool.tile([P, M], F32, tag="alpha")
        nc.vector.tensor_mul(alpha[:, :], alpha0[:, :], nwt[:, :])
        # b_m = -(S_m/C)*alpha_m = -(1 + sw_m/C)*alpha_m
        zz = stpool.tile([P, M], F32, tag="zz")
        nc.vector.tensor_scalar(zz[:, :], sw[:, :], 1.0 / C, 1.0, OP.mult, OP.add)
        bvals = stpool.tile([P, M], F32, tag="b")
        nc.vector.scalar_tensor_tensor(
            bvals[:, :], zz[:, :], -1.0, alpha[:, :], OP.mult, OP.mult
        )
        # Bias for the final Square: sum of b_m for m>=1 (b_0 folded into acc).
        bsum = stpool.tile([P, 1], F32, tag="bsum")
        nc.vector.reduce_sum(bsum[:, :], bvals[:, 1:M], axis=mybir.AxisListType.X)

        # s-hat accumulation chain
        acc = accpool.tile([P, C], F32, tag="acc")
        nc.vector.tensor_scalar(
            acc[:, :], xt[0][:, t, :], alpha[:, 0:1], bvals[:, 0:1], OP.mult, OP.add
        )
        for m in range(1, M):
            nacc = accpool.tile([P, C], F32, tag="acc")
            nc.vector.scalar_tensor_tensor(
                nacc[:, :], xt[m][:, t, :], alpha[:, m : m + 1], acc[:, :],
                OP.mult, OP.add,
            )
            acc = nacc

        sq = sqpool.tile([P, C], F32)
        nc.scalar.activation(
            sq[:, :], acc[:, :], AF.Square, bias=bsum[:, :],
            accum_out=ssq[:, t : t + 1],
        )
        if dbg is not None and t == 0:
            d_sv, d_qv, d_al, d_acc = dbg
            nc.sync.dma_start(d_sv[:, :], sw[:, :])
            nc.sync.dma_start(d_qv[:, :], tvar[:, :])
            nc.sync.dma_start(d_al[:, :], alpha[:, :])
            nc.sync.dma_start(d_acc[:, :], acc[:, :])

    nc.sync.dma_start(out[:, :], ssq[:, :])


def build_program(debug=False):
    nc = bacc.Bacc()
    xs = [
        nc.declare_dram_parameter(f"x{m}", [RPC, C], F32, isOutput=False)
        for m in range(M)
    ]
    out = nc.declare_dram_parameter("out", [P, NT], F32, isOutput=True)
    dbg = None
    if debug:
        dbg = (
            nc.declare_dram_parameter("d_sv", [P, M], F32, isOutput=True),
            nc.declare_dram_parameter("d_qv", [P, M], F32, isOutput=True),
            nc.declare_dram_parameter("d_al", [P, M], F32, isOutput=True),
            nc.declare_dram_parameter("d_acc", [P, C], F32, isOutput=True),
        )
    with tile.TileContext(nc) as tc:
        with ExitStack() as ctx:
            _body(ctx, tc, nc, xs, out, dbg)
    nc.compile()
    return nc


_prog = None


def kernel(**inputs):
    global _prog, LAST_RESULT
    xs_full = [
        np.ascontiguousarray(np.asarray(inputs[f"outputs{m + 1}"], dtype=np.float32))
        for m in range(M)
    ]
    if _prog is None:
        _prog = build_program(debug=DEBUG)
    core_ids = list(range(N_CORES))
    in_maps = [
        {f"x{m}": xs_full[m][k * RPC : (k + 1) * RPC] for m in range(M)}
        for k in core_ids
    ]
    res = run_bass_kernel_spmd(_prog, in_maps, core_ids, trace=TRACE)
    LAST_RESULT = res
    total = 0.0
    for r in res.results:
        total += np.asarray(r["out"], dtype=np.float64).sum()
    loss = SCALE * 0.5 * (total / B_TOTAL - M)
    return np.asarray(loss, dtype=np.float32)


# revision 37
# speedup vs baseline: 1.1238x; 1.0785x over previous
"""Trainium2 Bass kernel for nn_Diversity6 (pairwise-correlation diversity loss).

Math (per sample row b, per model m):
    e_m = exp(x_m / T)                      (softmax numerator; inputs are small,
                                             no max-subtraction needed)
    p_m = e_m / sum(e_m)
    u_m = (p_m - mean(p_m)) / ||p_m - mean(p_m)||
        = (e_m - mean(e_m)) / sqrt(C * var(e_m))      (sum(e) cancels!)
        = alpha_m * e_m + b_m
    with alpha_m = 1/sqrt(C*var(e_m)), b_m = -mean(e_m)*alpha_m.

    sum over all ordered pairs of corr(u_m, u_n) = ||sum_m u_m||^2, so
    d_b = (||s_b||^2 - M)/2 with s = sum_m u_m, and
    loss = SCALE * mean_b d_b.

Sharding: data-parallel over the batch dim, 512 rows per core on 8 cores.
Each core returns per-(partition, row-tile) values of ||s||^2; the host sums
them and applies the affine to produce the scalar loss.

Numerics: shifted moments (w = e - 1) avoid the catastrophic cancellation of
Q - S^2/C at |S|~1000 in fp32, and one Newton step repairs the ~2e-4 relative
error of the Ln/Exp LUT path for alpha = rsqrt(tvar).
"""

import math
from contextlib import ExitStack

import numpy as np

import concourse.bass as bass
import concourse.mybir as mybir
import concourse.tile as tile
from concourse import bacc
from concourse.bass_utils import run_bass_kernel_spmd

N_CORES = 8
B_TOTAL = 4096
C = 1000
M = 6
P = 128
RPC = B_TOTAL // N_CORES  # rows per core = 512
NT = RPC // P             # row-tiles per core = 4
T_INV = 1.0 / 20.0
SCALE = 0.3
NEG_HALF_LN_C = -0.5 * math.log(float(C))

F32 = mybir.dt.float32
AF = mybir.ActivationFunctionType
OP = mybir.AluOpType

TRACE = False
DEBUG = False
LAST_RESULT = None
N_QACT = 2  # how many models' qw runs on ACT (Square) vs DVE (stt)


def _body(ctx, tc, nc, xs, out, dbg=None):
    xv = [x.rearrange("(t p) c -> p t c", p=P) for x in xs]

    xpool = ctx.enter_context(tc.tile_pool(name="x", bufs=1))
    bnpool = ctx.enter_context(tc.tile_pool(name="bn", bufs=3))
    mvpool = ctx.enter_context(tc.tile_pool(name="mv", bufs=2))
    stpool = ctx.enter_context(tc.tile_pool(name="st", bufs=2))
    accpool = ctx.enter_context(tc.tile_pool(name="acc", bufs=3))
    sqpool = ctx.enter_context(tc.tile_pool(name="sq", bufs=2, space="PSUM"))
    opool = ctx.enter_context(tc.tile_pool(name="o", bufs=1))

    # Resident model tiles; 6 x 16KB/partition = 96KB/partition of SBUF.
    xt = [xpool.tile([P, NT, C], F32, tag=f"x{m}", name=f"x{m}sb") for m in range(M)]
    # Two 1MB DMAs per model, issued in the order compute consumes them.
    for h in range(2):
        for m in range(M):
            nc.sync.dma_start(
                xt[m][:, 2 * h : 2 * h + 2, :], xv[m][:, 2 * h : 2 * h + 2, :]
            )

    # Constant -1.0 bias column for the ACT Square(e-1) passes.
    negone = stpool.tile([P, 1], F32, tag="negone")
    nc.vector.memset(negone[:, :], -1.0)

    ssq = opool.tile([P, NT], F32)
    for t in range(NT):
        # Shifted moments, w = e - 1 (small!), to avoid the catastrophic
        # cancellation of Q - S^2/C at |S|~1000 in fp32:
        #   sw = sum(w);  qw = sum(w^2);  tvar = C*var(e) = qw - sw^2/C.
        sv = mvpool.tile([P, M], F32, tag="sv")
        qw = mvpool.tile([P, M], F32, tag="qw")
        for m in range(M):
            e = xt[m][:, t, :]  # [P, C]
            # S = sum(e) rides the exp pass's accumulator for free; sw = S - C
            # is accurate enough everywhere it is used (always scaled down).
            nc.scalar.activation(
                e, e, AF.Exp, scale=T_INV, accum_out=sv[:, m : m + 1]
            )
            if m < M - N_QACT:
                # qw holds sum((e-1)*e) = qw + sw for now; fixed below.
                esq = bnpool.tile([P, 1], F32, tag="esq")
                nc.vector.scalar_tensor_tensor(
                    esq.broadcast_to((P, C)), e, -1.0, e, OP.add, OP.mult,
                    accum_out=qw[:, m : m + 1],
                )
            else:
                # ACT path: Square(e - 1) accumulated = sum(w^2) directly.
                sqs = sqpool.tile([P, C], F32, tag="sqs")
                nc.scalar.activation(
                    sqs[:, :], e, AF.Square, bias=negone[:, :],
                    accum_out=qw[:, m : m + 1],
                )
        sw = mvpool.tile([P, M], F32, tag="sw")
        nc.vector.tensor_scalar(sw[:, :], sv[:, :], -float(C), None, OP.add)
        nq = M - N_QACT
        if nq > 0:
            # DVE-path columns hold sum(w*e) = qw + sw; subtract sw.
            nc.vector.tensor_sub(qw[:, 0:nq], qw[:, 0:nq], sw[:, 0:nq])

        # tvar = qw - sw^2/C
        tvar = stpool.tile([P, M], F32, tag="tvar")
        nc.vector.scalar_tensor_tensor(
            tvar[:, :], sw[:, :], -1.0 / C, sw[:, :], OP.mult, OP.mult
        )
        nc.vector.tensor_add(tvar[:, :], tvar[:, :], qw[:, :])
        lnv = stpool.tile([P, M], F32, tag="lnv")
        nc.scalar.activation(lnv[:, :], tvar[:, :], AF.Ln)
        alpha0 = stpool.tile([P, M], F32, tag="alpha0")
        nc.scalar.activation(alpha0[:, :], lnv[:, :], AF.Exp, scale=-0.5)
        # One Newton step for rsqrt: alpha = alpha0*(1.5 - 0.5*tvar*alpha0^2).
        # The Ln/Exp LUT path is only ~2e-4 accurate; this makes it fp32-exact.
        nwt = stpool.tile([P, M], F32, tag="nwt")
        nc.vector.tensor_mul(nwt[:, :], alpha0[:, :], alpha0[:, :])
        nc.vector.tensor_mul(nwt[:, :], nwt[:, :], tvar[:, :])
        nc.vector.tensor_scalar(nwt[:, :], nwt[:, :], -0.5, 1.5, OP.mult, OP.add)
        alpha = stpool.tile([P, M], F32, tag="alpha")
        nc.vector.tensor_mul(alpha[:, :], alpha0[:, :], nwt[:, :])
        # b_m = -(S_m/C)*alpha_m = -(1 + sw_m/C)*alpha_m
        zz = stpool.tile([P, M], F32, tag="zz")
        nc.vector.tensor_scalar(zz[:, :], sw[:, :], 1.0 / C, 1.0, OP.mult, OP.add)
        bvals = stpool.tile([P, M], F32, tag="b")
        nc.vector.scalar_tensor_tensor(
            bvals[:, :], zz[:, :], -1.0, alpha[:, :], OP.mult, OP.mult
        )
        # Bias for the final Square: sum of b_m for m>=1 (b_0 folded into acc).
        bsum = stpool.tile([P, 1], F32, tag="bsum")
        nc.vector.reduce_sum(bsum[:, :], bvals[:, 1:M], axis=mybir.AxisListType.X)

        # s-hat accumulation chain
        acc = accpool.tile([P, C], F32, tag="acc")
        nc.vector.tensor_scalar(
            acc[:, :], xt[0][:, t, :], alpha[:, 0:1], bvals[:, 0:1], OP.mult, OP.add
        )
        for m in range(1, M):
            nacc = accpool.tile([P, C], F32, tag="acc")
            nc.vector.scalar_tensor_tensor(
                nacc[:, :], xt[m][:, t, :], alpha[:, m : m + 1], acc[:, :],
                OP.mult, OP.add,
            )
            acc = nacc

        sq = sqpool.tile([P, C], F32)
        nc.scalar.activation(
            sq[:, :], acc[:, :], AF.Square, bias=bsum[:, :],
            accum_out=ssq[:, t : t + 1],
        )
        if dbg is not None and t == 0:
            d_sv, d_qv, d_al, d_acc = dbg
            nc.sync.dma_start(d_sv[:, :], sw[:, :])
            nc.sync.dma_start(d_qv[:, :], tvar[:, :])
            nc.sync.dma_start(d_al[:, :], alpha[:, :])
            nc.sync.dma_start(d_acc[:, :], acc[:, :])

    nc.sync.dma_start(out[:, :], ssq[:, :])


def build_program(debug=False):
    nc = bacc.Bacc()
    xs = [
        nc.declare_dram_parameter(f"x{m}", [RPC, C], F32, isOutput=False)
        for m in range(M)
    ]
    out = nc.declare_dram_parameter("out", [P, NT], F32, isOutput=True)
    dbg = None
    if debug:
        dbg = (
            nc.declare_dram_parameter("d_sv", [P, M], F32, isOutput=True),
            nc.declare_dram_parameter("d_qv", [P, M], F32, isOutput=True),
            nc.declare_dram_parameter("d_al", [P, M], F32, isOutput=True),
            nc.declare_dram_parameter("d_acc", [P, C], F32, isOutput=True),
        )
    with tile.TileContext(nc) as tc:
        with ExitStack() as ctx:
            _body(ctx, tc, nc, xs, out, dbg)
    nc.compile()
    return nc


_prog = None


def kernel(**inputs):
    global _prog, LAST_RESULT
    xs_full = [
        np.ascontiguousarray(np.asarray(inputs[f"outputs{m + 1}"], dtype=np.float32))
        for m in range(M)
    ]
    if _prog is None:
        _prog = build_program(debug=DEBUG)
    core_ids = list(range(N_CORES))
    in_maps = [
        {f"x{m}": xs_full[m][k * RPC : (k + 1) * RPC] for m in range(M)}
        for k in core_ids
    ]
    res = run_bass_kernel_spmd(_prog, in_maps, core_ids, trace=TRACE)
    LAST_RESULT = res
    total = 0.0
    for r in res.results:
        total += np.asarray(r["out"], dtype=np.float64).sum()
    loss = SCALE * 0.5 * (total / B_TOTAL - M)
    return np.asarray(loss, dtype=np.float32)
